# revision 7
# baseline (speedup 1.0000x reference)
"""BERT encoder forward pass on 8 TRN2 NeuronCores.

Strategy: pure data parallelism over the batch (16 sequences -> 2 per core).
Each core runs the full 12-layer encoder on its 2 sequences; no collectives.

Device layout (per core, T = 2*512 = 1024 tokens, L = 512 per sequence):
  h_t  : residual accumulator, fp32, token-major      [t(8x128 part), d(768)]
  hb   : LayerNorm output, bf16, token-major          [t part, d]
  hT_s : per-sequence transposed LN output, bf16      [d(128 part), dchunk(6), t(512)]
         produced by an xbar DMA transpose through a DRAM staging buffer
         (frees the tensor engine from 96 transposes/layer). Attention
         context overwrites hT_s in place (WAR-tracked) so Wo reads it as
         the transposed context.
  qp/kp: q^T / k^T per (seq, head-pair), bf16         [128 part = 2 heads x 64dk,
         512 free = positions]; scores run as TWO row-tiled matmuls
         (tile_position (0,0) / (64,0)) that execute concurrently on
         disjoint PE row groups -> 2x score throughput.
  v    : bf16 token-major with a mask column per head [t part, 12*(64+1)]
         col 64 of each head = key-validity (1/0). The ctx matmul over-reads
         a 128-wide stationary slice (FWL -> hidden LDWEIGHTS); psum row 64
         is the softmax denominator, rows 65+ garbage. Masked keys have
         their v rows AND mask column zeroed, so no exp-bias masking is
         needed: softmax numerator/denominator both exclude them exactly.
All matmuls contract over the partition dim with fp32 PSUM accumulation.
Softmax skips max-subtraction (scores are O(1)).
"""

import os

import ml_dtypes
import numpy as np

B, L, D, NL, H, DK, FF = 16, 512, 768, 12, 12, 64, 3072
NCORES = 8
SPC = B // NCORES          # sequences per core
T = SPC * L                # tokens per core
DCH = D // 128             # 6 chunks of d
FCH = FF // 128            # 24 chunks of ff
TCH = T // 128             # 8 chunks of t
NP = H // 2                # 6 head pairs
VW = DK + 1                # 65: per-head v width (dk + mask col)
DH = D // 2                # 384: FF2 half width

_CACHE = {}


def _build_program(nl, use_bias, use_affine):
    import concourse.mybir as mybir
    import concourse.tile as tile
    from concourse import bacc

    f32, bf16 = mybir.dt.float32, mybir.dt.bfloat16
    AF = mybir.ActivationFunctionType
    OP = mybir.AluOpType

    nc = bacc.Bacc("TRN2", target_bir_lowering=False, debug=False)
    h0_d = nc.dram_tensor("h0", [T, D], f32, kind="ExternalInput").ap()
    h0T_d = nc.dram_tensor("h0T", [SPC, D, L], bf16, kind="ExternalInput").ap()
    maskf_d = nc.dram_tensor("maskf", [T, 1], f32, kind="ExternalInput").ap()
    wq_d = nc.dram_tensor("wq", [nl, D, D], bf16, kind="ExternalInput").ap()
    wk_d = nc.dram_tensor("wk", [nl, D, D], bf16, kind="ExternalInput").ap()
    wv_d = nc.dram_tensor("wv", [nl, D, D], bf16, kind="ExternalInput").ap()
    wo_d = nc.dram_tensor("wo", [nl, D, D], bf16, kind="ExternalInput").ap()
    w1_d = nc.dram_tensor("w1", [nl, D, FF], bf16, kind="ExternalInput").ap()
    w2_d = nc.dram_tensor("w2", [nl, FF, D], bf16, kind="ExternalInput").ap()
    names = ["h0", "h0T", "maskf", "wq", "wk", "wv", "wo", "w1", "w2"]
    if use_bias:
        bqk_d = nc.dram_tensor("bqk", [nl, 2, D], f32, kind="ExternalInput").ap()
        b1_d = nc.dram_tensor("b1", [nl, FF], f32, kind="ExternalInput").ap()
        brow_d = nc.dram_tensor("brow", [nl, 3, D], bf16, kind="ExternalInput").ap()
        names += ["bqk", "b1", "brow"]
    if use_affine:
        lng_d = nc.dram_tensor("lng", [nl, D], f32, kind="ExternalInput").ap()
        lnb_d = nc.dram_tensor("lnb", [nl, D], f32, kind="ExternalInput").ap()
        names += ["lng", "lnb"]
    out_d = nc.dram_tensor("out", [T, D], f32, kind="ExternalOutput").ap()

    with tile.TileContext(nc) as tc:
        with (
            tc.tile_pool(name="const", bufs=1) as cp,
            tc.tile_pool(name="persist", bufs=1) as pp,
            tc.tile_pool(name="wts", bufs=1) as wp,
            tc.tile_pool(name="work", bufs=2) as wk,
            tc.tile_pool(name="stage", bufs=2, space="DRAM") as dp,
            tc.tile_pool(name="psum", bufs=2, space="PSUM") as psp,
        ):
            eps_t = cp.tile([128, 1], f32)
            nc.vector.memset(eps_t[:], 1e-5)
            mtile = cp.tile([128, TCH], f32)
            nc.sync.dma_start(mtile[:], maskf_d.rearrange("(i p) o -> p (i o)", p=128))
            if use_bias:
                ones_row = cp.tile([1, 128], bf16)
                nc.vector.memset(ones_row[:], 1.0)

            h_t = [pp.tile([128, D], f32, tag=f"h{i}", name=f"h{i}") for i in range(TCH)]
            hb = [pp.tile([128, D], bf16, tag=f"hb{i}", name=f"hb{i}") for i in range(TCH)]
            hT_s = [pp.tile([128, DCH, L], bf16, tag=f"hT{s}", name=f"hT{s}")
                    for s in range(SPC)]
            qp = [[pp.tile([128, L], bf16, tag=f"qp{s}_{p}", name=f"qp{s}_{p}")
                   for p in range(NP)] for s in range(SPC)]
            kp = [[pp.tile([128, L], bf16, tag=f"kp{s}_{p}", name=f"kp{s}_{p}")
                   for p in range(NP)] for s in range(SPC)]
            # v tiles padded to 844 so the 128-wide over-read at head 11
            # (offset 715) stays in bounds; pad cols stay zero.
            v_t = [pp.tile([128, H * VW + DK], bf16, tag=f"v{i}", name=f"v{i}")
                   for i in range(TCH)]
            gel = [pp.tile([128, T], bf16, tag=f"g{f}", name=f"g{f}") for f in range(FCH)]

            # one-time init: load h/hT from host, set v mask columns
            for i in range(TCH):
                nc.sync.dma_start(h_t[i][:], h0_d[i * 128:(i + 1) * 128, :])
                nc.vector.memset(v_t[i][:], 0.0)
                v65 = v_t[i][:, 0:H * VW].rearrange("p (h e) -> p h e", e=VW)
                nc.vector.memset(v65[:, :, DK:DK + 1], 1.0)
                nc.vector.tensor_scalar(v65[:, :, DK:DK + 1], v65[:, :, DK:DK + 1],
                                        mtile[:, i:i + 1], None, op0=OP.mult)
            for s in range(SPC):
                nc.sync.dma_start(
                    hT_s[s][:],
                    h0T_d[s].rearrange("(c p) t -> p c t", p=128))

            def layernorm(i, gb, last):
                st = wk.tile([128, 2, 6], f32, tag="bnst", bufs=2)
                for g in range(2):
                    nc.vector.bn_stats(st[:, g, :], h_t[i][:, g * 384:(g + 1) * 384])
                mv = wk.tile([128, 2], f32, tag="bnmv", bufs=2)
                nc.vector.bn_aggr(mv[:], st[:])
                rstd = wk.tile([128, 1], f32, tag="rstd", bufs=2)
                nc.scalar.activation(rstd[:], mv[:, 1:2], AF.Sqrt, bias=eps_t[:])
                nc.vector.reciprocal_approx_fast(rstd[:], rstd[:])
                nc.vector.tensor_scalar(h_t[i][:], h_t[i][:], mv[:, 0:1], rstd[:],
                                        op0=OP.subtract, op1=OP.mult)
                if gb is not None:
                    nc.vector.tensor_tensor(h_t[i][:], h_t[i][:], gb[0][:], op=OP.mult)
                    nc.vector.tensor_tensor(h_t[i][:], h_t[i][:], gb[1][:], op=OP.add)
                if not last:
                    nc.vector.tensor_copy(hb[i][:], h_t[i][:])

            def retranspose():
                # hb -> DRAM staging -> xbar-transposed reload into hT_s
                hst = dp.tile([T, D], bf16, tag="hst")
                for i in range(TCH):
                    nc.sync.dma_start(hst[i * 128:(i + 1) * 128, :], hb[i][:])
                for s in range(SPC):
                    nc.scalar.dma_start_transpose(
                        hT_s[s][:], hst[s * L:(s + 1) * L, :])

            for l in range(nl):
                if use_bias:
                    bqk_sb = wk.tile([128, 2 * DCH], f32, tag="bqk")
                    nc.sync.dma_start(bqk_sb[:],
                                      bqk_d[l].rearrange("b (c p) -> p (b c)", p=128))
                    b1_sb = wk.tile([128, FCH], f32, tag="b1sb")
                    nc.sync.dma_start(b1_sb[:],
                                      b1_d[l].rearrange("(c p) -> p c", p=128))
                    brow_sb = wk.tile([3, D], bf16, tag="brow")
                    nc.sync.dma_start(brow_sb[:], brow_d[l])
                if use_affine:
                    g_bc = wk.tile([128, D], f32, tag="gbc")
                    b_bc = wk.tile([128, D], f32, tag="bbc")
                    nc.gpsimd.dma_start(g_bc[:], lng_d[l:l + 1, :].to_broadcast((128, D)))
                    nc.gpsimd.dma_start(b_bc[:], lnb_d[l:l + 1, :].to_broadcast((128, D)))
                    gb = (g_bc, b_bc)
                else:
                    gb = None

                # ---- q^T / k^T into per-(seq, head-pair) tiles ----
                for mat_d, dstp, bcol in ((wq_d, qp, 0), (wk_d, kp, 1)):
                    for dc in range(DCH):
                        wc = wk.tile([128, DCH, 128], bf16, tag="wqkcol", bufs=3,
                                     name="wqkc")
                        nc.sync.dma_start(
                            wc[:], mat_d[l][:, dc * 128:(dc + 1) * 128]
                            .rearrange("(c p) n -> p c n", p=128))
                        ps = [psp.tile([128, L], f32, tag="p5", bufs=4, name="psqk")
                              for _ in range(SPC)]
                        for c in range(DCH):
                            for s in range(SPC):
                                nc.tensor.matmul(ps[s][:], wc[:, c, :],
                                                 hT_s[s][:, c, :],
                                                 start=(c == 0), stop=(c == DCH - 1))
                        for s in range(SPC):
                            if use_bias:
                                nc.scalar.activation(
                                    dstp[s][dc][:], ps[s][:], AF.Identity,
                                    bias=bqk_sb[:, bcol * DCH + dc:bcol * DCH + dc + 1])
                            else:
                                nc.vector.tensor_copy(dstp[s][dc][:], ps[s][:])

                # ---- v (token-major, masked, with mask cols) ----
                wv_rows = []
                for c in range(DCH):
                    wr = wp.tile([128, D], bf16, tag=f"wv{c}", name=f"wv{c}")
                    nc.sync.dma_start(wr[:], wv_d[l, c * 128:(c + 1) * 128, :])
                    wv_rows.append(wr)
                for i in range(TCH):
                    s, j = i // 4, i % 4
                    psA = psp.tile([128, L], f32, tag="p5", bufs=4)
                    psB = psp.tile([128, 256], f32, tag="p5", bufs=4)
                    for c in range(DCH):
                        stat = hT_s[s][:, c, j * 128:(j + 1) * 128]
                        nc.tensor.matmul(psA[:], stat, wv_rows[c][:, 0:512],
                                         start=(c == 0),
                                         stop=(c == DCH - 1 and not use_bias))
                        nc.tensor.matmul(psB[:], stat, wv_rows[c][:, 512:768],
                                         start=(c == 0),
                                         stop=(c == DCH - 1 and not use_bias))
                    if use_bias:
                        nc.tensor.matmul(psA[:], ones_row[:], brow_sb[0:1, 0:512],
                                         start=False, stop=True)
                        nc.tensor.matmul(psB[:], ones_row[:], brow_sb[0:1, 512:768],
                                         start=False, stop=True)
                    v65 = v_t[i][:, 0:H * VW].rearrange("p (h e) -> p h e", e=VW)
                    nc.vector.tensor_scalar(
                        v65[:, 0:8, 0:DK], psA[:].rearrange("p (h e) -> p h e", e=DK),
                        mtile[:, i:i + 1], None, op0=OP.mult)
                    nc.vector.tensor_scalar(
                        v65[:, 8:12, 0:DK], psB[:].rearrange("p (h e) -> p h e", e=DK),
                        mtile[:, i:i + 1], None, op0=OP.mult)

                # ---- attention per (seq, head pair): row-tiled scores ----
                for s in range(SPC):
                    for p in range(NP):
                        nm = [[None, None], [None, None]]
                        for half in range(2):
                            sc = [psp.tile([128, T], f32, tag="sc", bufs=2, name="sct")
                                  for _ in range(2)]
                            for q in range(2):
                                tk = 2 * half + q
                                for hh in range(2):
                                    nc.tensor.matmul(
                                        sc[hh][:, q * L:(q + 1) * L],
                                        kp[s][p][hh * 64:(hh + 1) * 64,
                                                 tk * 128:(tk + 1) * 128],
                                        qp[s][p][hh * 64:(hh + 1) * 64, :],
                                        start=True, stop=True)
                            for hh in range(2):
                                t_nm = wk.tile([128, T], bf16, tag="numer", bufs=4,
                                               name="nm")
                                nc.scalar.activation(t_nm[:], sc[hh][:], AF.Exp)
                                nm[hh][half] = t_nm
                        for hh in range(2):
                            hd = 2 * p + hh
                            cps = psp.tile([128, L], f32, tag="p5", bufs=4)
                            for tk in range(4):
                                nc.tensor.matmul(
                                    cps[:], v_t[s * 4 + tk][:, hd * VW:hd * VW + 128],
                                    nm[hh][tk // 2][:, (tk % 2) * L:(tk % 2 + 1) * L],
                                    start=(tk == 0), stop=(tk == 3))
                            rec = wk.tile([1, L], f32, tag="rec", bufs=2)
                            nc.vector.tensor_copy(rec[:], cps[DK:DK + 1, :])
                            nc.vector.reciprocal_approx_fast(rec[:], rec[:])
                            recb = wk.tile([DK, L], f32, tag="recb", bufs=2)
                            nc.gpsimd.partition_broadcast(recb[:], rec[:])
                            dst = hT_s[s][hh * 64:hh * 64 + 64, p, :]
                            nc.vector.tensor_tensor(dst, cps[0:DK, :], recb[:],
                                                    op=OP.mult)

                # ---- attn output + residual (vs hb) ----
                wo_rows = []
                for c in range(DCH):
                    wr = wp.tile([128, D], bf16, tag=f"wo{c}", name=f"wo{c}")
                    nc.sync.dma_start(wr[:], wo_d[l, c * 128:(c + 1) * 128, :])
                    wo_rows.append(wr)
                for i in range(TCH):
                    s, j = i // 4, i % 4
                    psA = psp.tile([128, L], f32, tag="p5", bufs=4)
                    psB = psp.tile([128, 256], f32, tag="p5", bufs=4)
                    for c in range(DCH):
                        stat = hT_s[s][:, c, j * 128:(j + 1) * 128]
                        nc.tensor.matmul(psA[:], stat, wo_rows[c][:, 0:512],
                                         start=(c == 0),
                                         stop=(c == DCH - 1 and not use_bias))
                        nc.tensor.matmul(psB[:], stat, wo_rows[c][:, 512:768],
                                         start=(c == 0),
                                         stop=(c == DCH - 1 and not use_bias))
                    if use_bias:
                        nc.tensor.matmul(psA[:], ones_row[:], brow_sb[1:2, 0:512],
                                         start=False, stop=True)
                        nc.tensor.matmul(psB[:], ones_row[:], brow_sb[1:2, 512:768],
                                         start=False, stop=True)
                    nc.vector.tensor_tensor(h_t[i][:, 0:512], psA[:],
                                            h_t[i][:, 0:512], op=OP.add)
                    nc.vector.tensor_tensor(h_t[i][:, 512:768], psB[:],
                                            h_t[i][:, 512:768], op=OP.add)
                    layernorm(i, gb, last=False)
                retranspose()

                # ---- ffn1 + gelu (transposed output) ----
                for f in range(FCH):
                    w1c = wk.tile([128, DCH, 128], bf16, tag="w1col", bufs=3,
                                  name="w1c")
                    nc.sync.dma_start(
                        w1c[:], w1_d[l][:, f * 128:(f + 1) * 128]
                        .rearrange("(c p) n -> p c n", p=128))
                    ps = [psp.tile([128, L], f32, tag="p5", bufs=4, name="psf1")
                          for _ in range(SPC)]
                    for c in range(DCH):
                        for g in range(SPC):
                            nc.tensor.matmul(ps[g][:], w1c[:, c, :],
                                             hT_s[g][:, c, :],
                                             start=(c == 0), stop=(c == DCH - 1))
                    for g in range(SPC):
                        nc.scalar.activation(
                            gel[f][:, g * L:(g + 1) * L], ps[g][:], AF.Gelu,
                            bias=(b1_sb[:, f:f + 1] if use_bias else 0.0))

                # ---- ffn2 + residual + LN2 ----
                for dh in range(2):
                    w2_rows = []
                    for f in range(FCH):
                        wr = wp.tile([128, DH], bf16, tag=f"w2r{f}", name=f"w2r{f}")
                        nc.sync.dma_start(
                            wr[:], w2_d[l, f * 128:(f + 1) * 128,
                                        dh * DH:(dh + 1) * DH])
                        w2_rows.append(wr)
                    for i in range(TCH):
                        ps = psp.tile([128, DH], f32, tag="p5", bufs=4)
                        for f in range(FCH):
                            nc.tensor.matmul(ps[:], gel[f][:, i * 128:(i + 1) * 128],
                                             w2_rows[f][:],
                                             start=(f == 0),
                                             stop=(f == FCH - 1 and not use_bias))
                        if use_bias:
                            nc.tensor.matmul(ps[:], ones_row[:],
                                             brow_sb[2:3, dh * DH:(dh + 1) * DH],
                                             start=False, stop=True)
                        nc.vector.tensor_tensor(h_t[i][:, dh * DH:(dh + 1) * DH],
                                                ps[:], h_t[i][:, dh * DH:(dh + 1) * DH],
                                                op=OP.add)
                        if dh == 1:
                            layernorm(i, gb, last=(l == nl - 1))
                if l < nl - 1:
                    retranspose()

            for i in range(TCH):
                nc.sync.dma_start(out_d[i * 128:(i + 1) * 128, :], h_t[i][:])

    nc.compile()
    return nc, names


def _get_program(nl, use_bias, use_affine):
    key = (nl, use_bias, use_affine)
    if key not in _CACHE:
        _CACHE[key] = _build_program(nl, use_bias, use_affine)
    return _CACHE[key]


def kernel(**inputs):
    from concourse import bass_utils

    x = np.asarray(inputs["x"])
    tok = np.asarray(inputs["token_emb"], np.float32)
    pe = np.asarray(inputs["pe"], np.float32)
    to_bf = lambda a: np.asarray(a, np.float32).astype(ml_dtypes.bfloat16)

    h0 = tok[x] + pe[None]                                   # (B, L, D) f32
    maskf = (x > 0).astype(np.float32)                       # (B, L)

    nl = int(os.environ.get("BERT_NL", str(NL)))
    bias_arrs = [np.asarray(inputs[k], np.float32)[:nl]
                 for k in ("bq", "bk", "bv", "bo", "b1", "b2")]
    use_bias = any(np.any(a != 0.0) for a in bias_arrs)
    lng = np.asarray(inputs["ln_g"], np.float32)[:nl]
    lnb = np.asarray(inputs["ln_b"], np.float32)[:nl]
    use_affine = bool(np.any(lng != 1.0) or np.any(lnb != 0.0))

    nc, names = _get_program(nl, use_bias, use_affine)

    shared = {
        "wq": to_bf(np.asarray(inputs["Wq"][:nl], np.float32) * 0.125),
        "wk": to_bf(inputs["Wk"][:nl]),
        "wv": to_bf(inputs["Wv"][:nl]), "wo": to_bf(inputs["Wo"][:nl]),
        "w1": to_bf(inputs["W1"][:nl]), "w2": to_bf(inputs["W2"][:nl]),
    }
    if use_bias:
        bq, bk, bv, bo, b1, b2 = bias_arrs
        shared["bqk"] = np.stack([bq * 0.125, bk], axis=1).astype(np.float32)
        shared["b1"] = b1.astype(np.float32)
        shared["brow"] = to_bf(np.stack([bv, bo, b2], axis=1))
    if use_affine:
        shared["lng"] = lng
        shared["lnb"] = lnb

    in_maps = []
    for c in range(NCORES):
        im = dict(shared)
        hc = h0[SPC * c:SPC * (c + 1)]                       # (SPC, L, D)
        im["h0"] = np.ascontiguousarray(
            hc.reshape(T, D), dtype=np.float32)
        im["h0T"] = np.ascontiguousarray(
            hc.transpose(0, 2, 1)).astype(ml_dtypes.bfloat16)
        im["maskf"] = np.ascontiguousarray(
            maskf[SPC * c:SPC * (c + 1)].reshape(T, 1), dtype=np.float32)
        in_maps.append(im)

    trace = os.environ.get("BERT_TRACE", "0") == "1"
    res = bass_utils.run_bass_kernel_spmd(
        nc, in_maps, core_ids=list(range(NCORES)), trace=trace)
    if trace:
        print(f"HW exec time: {res.exec_time_ns} ns")
        try:
            import pickle
            insts, tpath = res.instructions_and_trace
            rows = [(i.engine, i.name, "", i.timestamp,
                     i.duration) for i in insts]
            with open("/root/problem/work/insts.pkl", "wb") as f:
                pickle.dump({"rows": rows, "trace_path": tpath,
                             "exec_time_ns": res.exec_time_ns}, f)
            print(f"trace dumped: {len(rows)} insts, {tpath}")
        except Exception as e:
            print("trace dump failed:", e)

    out = np.stack([np.asarray(res.results[c]["out"]).reshape(SPC, L, D)
                    for c in range(NCORES)])
    return out.reshape(B, L, D).astype(np.float32)
